# revision 25
# baseline (speedup 1.0000x reference)
import numpy as np

TAU = 10.0
THR = 1.0
ALPHA = float(np.exp(-1.0 / TAU))
B, T = 128, 100
NCORES = 8
BL = B // NCORES  # 16 samples per core
BT = BL * T       # 1600 images per core
NBLK = 4
BLK = BT // NBLK  # 400 images (4 samples) per block

# conv matmul mode: "f32r" (full speed, reduced precision) or "f32" (4x
# slower, exact). The net is chaotic (spike-time cascades amplify noise
# ~noise^0.46), so conv precision must be >=17 mantissa bits.
CONV_MODE = "f32"


def _fused_mats(conv1_w, conv2_w):
    """Fold avgpool into conv -> dense per-image matrices.
    M1: [256=(o,H,W), 2048=(i,h,w)]  (conv1 5x5 pad2 + pool4 -> 8x8 stride4 kernel, pad 2)
    M2: [128=(o2,H2,W2), 256=(i2,h,w)] (conv2 3x3 pad1 + pool2 -> 4x4 stride2 kernel, pad 1)
    """
    w1 = conv1_w.astype(np.float64)
    W1f = np.zeros((4, 2, 8, 8))
    for dh in range(4):
        for dw in range(4):
            W1f[:, :, dh:dh + 5, dw:dw + 5] += w1
    W1f /= 16.0
    M1 = np.zeros((4, 8, 8, 2, 32, 32))
    for Ho in range(8):
        for Wo in range(8):
            for a in range(8):
                h = 4 * Ho + a - 2
                if not (0 <= h < 32):
                    continue
                for b in range(8):
                    w = 4 * Wo + b - 2
                    if 0 <= w < 32:
                        M1[:, Ho, Wo, :, h, w] = W1f[:, :, a, b]
    M1 = M1.reshape(256, 2048).astype(np.float32)

    w2 = conv2_w.astype(np.float64)
    W2f = np.zeros((8, 4, 4, 4))
    for dh in range(2):
        for dw in range(2):
            W2f[:, :, dh:dh + 3, dw:dw + 3] += w2
    W2f /= 4.0
    M2 = np.zeros((8, 4, 4, 4, 8, 8))
    for Ho in range(4):
        for Wo in range(4):
            for a in range(4):
                h = 2 * Ho + a - 1
                if not (0 <= h < 8):
                    continue
                for b in range(4):
                    w = 2 * Wo + b - 1
                    if 0 <= w < 8:
                        M2[:, Ho, Wo, :, h, w] = W2f[:, :, a, b]
    M2 = M2.reshape(128, 256).astype(np.float32)
    return M1, M2


def _numpy_forward(x, conv1_w, conv2_w, lin_w):
    M1, M2 = _fused_mats(conv1_w, conv2_w)
    X = x.reshape(B * T, 2048).astype(np.float32)
    U1 = (X @ M1.T).reshape(B, T, 256)

    def leak_lif(U):  # U: [B,T,F] -> spikes [B,T,F]
        Bb, Tt, F = U.shape
        u = np.zeros((Bb, F), np.float32)
        v = np.zeros((Bb, F), np.float32)
        S = np.zeros_like(U)
        for t in range(Tt):
            u = ALPHA * u + U[:, t]
            v = ALPHA * v + u
            s = (v >= THR).astype(np.float32)
            v = v - s * THR
            S[:, t] = s
        return S

    S1 = leak_lif(U1)
    U2 = (S1.reshape(B * T, 256) @ M2.T).reshape(B, T, 128)
    S2 = leak_lif(U2)
    return (S2.reshape(B * T, 128) @ lin_w.T.astype(np.float32)).reshape(B, T, 2)


# conv1 band decomposition: output features are stored band-permuted
# (fnew = (Ho//4)*128 + (Ho%4)*32 + o*8 + Wo); each x k-chunk then feeds
# only one/two 128-row output halves, so conv1 is 20 matmuls per image
# block instead of 32. The permutation is absorbed into M2's column
# order on the host. BANDS is structural (conv geometry), not data.
BANDS = [(0, 0), (0, 1), (0, 2), (0, 3), (0, 4),
         (0, 8), (0, 9), (0, 10), (0, 11), (0, 12),
         (1, 3), (1, 4), (1, 5), (1, 6), (1, 7),
         (1, 11), (1, 12), (1, 13), (1, 14), (1, 15)]
M2OFF = 20 * 128
LINOFF = M2OFF + 256
WCOLS = LINOFF + 2


# x chunk order: the 10 chunks used by the j2=0 psum half come first, so
# conv1's first matmul group only waits for the first ~2MB of each quarter
CH_ORDER = [0, 1, 2, 3, 4, 8, 9, 10, 11, 12, 5, 6, 7, 13, 14, 15]
CH_POS = {c: i for i, c in enumerate(CH_ORDER)}


def _band_perm():
    perm = np.empty(256, np.int64)
    for Ho in range(8):
        for o in range(4):
            for Wo in range(8):
                fnew = (Ho // 4) * 128 + (Ho % 4) * 32 + o * 8 + Wo
                perm[fnew] = o * 64 + Ho * 8 + Wo
    return perm



_LIF_OP = None


def _get_lif_op():
    """Register a custom DVE op computing one LIF step in a single
    instruction:  out = (in0 + (in0 <= s1)) * s0 - in1
    (in0 = previous w_pre, in1 = leaked drive v[t], s0 = alpha, s1 = -1).
    4 ALU stages; replaces two chained scalar_tensor_tensor ops."""
    global _LIF_OP
    if _LIF_OP is not None:
        return _LIF_OP
    import numpy as np
    import concourse.dve_ops as dve_ops
    from concourse.dve_spec import Spec, Src0, Src1, C0, C1, lower, _has_src1
    from concourse.dve_uop import DveOpSpec

    name = "LIF_STEP_ANT"
    spec = Spec(
        body=(Src0 + (Src0 <= C1)) * C0 - Src1,
        reference=lambda in0, in1, s0, s1, imm2:
            (in0 + (in0 <= s1)) * s0 - in1,
    )
    row = dve_ops._CUSTOM_DVE_ROW_BASE + len(dve_ops.OPS)
    shas = {}
    for ver in ("v3", "v4"):
        try:
            dspec = DveOpSpec(name=name, opcode=row,
                              uops=lower(spec, ver=ver),
                              rd1_en=_has_src1(spec))
            shas[ver] = dspec.sha(ver)
        except Exception:
            pass
    op = dve_ops.DveOp(name, spec, subdim=False, uops_sha=shas)
    dve_ops.OPS.append(op)
    dve_ops._SUB_OPCODE_FOR_NAME[name] = row
    dve_ops.CUSTOM_DVE_SPECS[name] = spec
    _LIF_OP = op
    return op


def _build_bass():
    import concourse.mybir as mybir
    from concourse.bacc import Bacc
    from concourse.tile import TileContext

    f32 = mybir.dt.float32
    cdt = mybir.dt.float32r if CONV_MODE == "f32r" else mybir.dt.float32
    Al = mybir.AluOpType
    # Bacc (not bass.Bass): its finalize() runs move_matmul_waits_to_ldweights
    # + generate_event_semaphores, which legalize multi-sem waits down to the
    # 1-wait-per-instruction limit walrus enforces.
    lif_op = _get_lif_op()
    nc = Bacc()
    # x quarters: [4][p(k%128), c(k//128), img(400)] f32 — small enough that
    # conv1 on block 0 starts ~10us in
    xt = nc.dram_tensor("xt", [NBLK, 128, 16, BLK], cdt, kind="ExternalInput")
    # all weights packed: [p, 16*256 (M1T) + 2*128 (M2T) + 2 (linT)] f32
    wts = nc.dram_tensor("wts", [128, WCOLS], cdt, kind="ExternalInput")
    out = nc.dram_tensor("out", [2, BT], f32, kind="ExternalOutput")

    with TileContext(nc) as tc:
        with (
            tc.tile_pool(name="big", bufs=1) as big,
            tc.tile_pool(name="xp", bufs=2) as xp,
            tc.tile_pool(name="pp", bufs=4, space="PSUM") as pp,
            tc.tile_pool(name="pl", bufs=2, space="PSUM") as pl,
        ):
            wsb = big.tile([128, WCOLS], cdt, name="wsb")
            nc.sync.dma_start(wsb[:], wts[:, :])

            xq = []
            for j in range(NBLK):
                tl = xp.tile([128, 16 * BLK], cdt, name="xblk")
                t3 = tl[:].rearrange("p (c n) -> p c n", n=BLK)
                nc.sync.dma_start(t3[:, 0:10, :], xt[j][:, 0:10, :])
                nc.sync.dma_start(t3[:, 10:16, :], xt[j][:, 10:16, :])
                xq.append(tl)

            # alpha pattern for leak scans: ALPHA everywhere, 0 at t%T==0
            # (DVE-built; the scans run on DVE so this is a same-engine dep)
            al = big.tile([128, BLK], f32, name="alpha")
            nc.vector.memset(al[:], ALPHA)
            al3 = al[:].rearrange("p (b t) -> p b t", t=T)
            nc.vector.memset(al3[:, :, 0], 0.0)

            # time-major state tiles: column index = t*LANES + lane, so the
            # per-step LIF slices are contiguous [128, LANES] (fast DVE path)
            v1 = big.tile([128, 2 * BT], f32, name="v1")    # [p, (t hb)]
            wp1 = big.tile([128, 2 * BT], f32, name="wp1")  # [p, (t hb)]
            s1 = big.tile([128, 2 * BT], cdt, name="s1")    # [p, (t hb)]
            v2 = big.tile([128, BT], f32, name="v2")        # [p, (t b)]
            wp2 = big.tile([128, BT], f32, name="wp2")      # [p, (t b)]
            s2 = big.tile([128, BT], cdt, name="s2")        # [p, (t b)]
            outsb = big.tile([2, BT], f32, name="outsb")

            # dummy PSUM tile: a tiny throwaway matmul absorbs each x-DMA
            # wait so real matmuls carry at most one sync wait each.
            dps = pl.tile([1, 8], f32, name="dps", bufs=1)

            # b-major scan landing pads (scan needs 2D APs); ACT then
            # relayouts them into the t-major state tiles
            vtmp_pool = xp  # reuse pool object only for namespacing

            # [p, hb(32) stride 1, t(100) stride 32] view of t-major layer-1
            v1bt = v1[:].rearrange("p (t hb) -> p hb t", hb=32)
            v2bt = v2[:].rearrange("p (t b) -> p b t", b=BL)

            # ---- conv1: dense fused conv+pool, 16-chunk contraction ----
            for bi in range(NBLK):
                xj = xq[bi][:, 0:1].bitcast(f32)
                nc.tensor.matmul(dps[:1, 0:1], xj, xj, start=True, stop=True)
                for h in range(2):
                    sub = [(i, c) for i, (jj, c) in enumerate(BANDS)
                           if jj == h]
                    ps = pp.tile([128, BLK], f32, name="cps")
                    for n, (idx, c) in enumerate(sub):
                        nc.tensor.matmul(
                            ps[:],
                            wsb[:, idx * 128:(idx + 1) * 128],
                            xq[bi][:, CH_POS[c] * BLK:
                                   (CH_POS[c] + 1) * BLK],
                            start=(n == 0), stop=(n == len(sub) - 1))
                    # leak scan straight out of PSUM (per-sample reset via
                    # the alpha=0 columns) into a b-major temp, then ACT
                    # relayouts into t-major v1
                    vt = big.tile([128, BLK], f32, name="vtmp", bufs=4)
                    nc.vector.tensor_tensor_scan(
                        vt[:], al[:], ps[:], 0.0, Al.mult, Al.add)
                    vdst = v1bt[:, h * 16 + 4 * bi: h * 16 + 4 * bi + 4, :]
                    nc.scalar.copy(
                        vdst, vt[:].rearrange("p (b t) -> p b t", t=T))

            # ---- LIF layer 1: one fused custom-DVE op per step ----
            z0 = big.tile([128, 32], f32, name="z0")
            nc.vector.memset(z0[:], 0.0)
            for t in range(T):
                nc.vector._custom_dve(
                    lif_op,
                    out=wp1[:, t * 32:(t + 1) * 32],
                    in0=(z0[:] if t == 0 else wp1[:, (t - 1) * 32:t * 32]),
                    in1=v1[:, t * 32:(t + 1) * 32],
                    s0=ALPHA, s1=-1.0)
            # spikes: s = (w_pre <= -1), one contiguous bulk op
            nc.vector.tensor_scalar(s1[:], wp1[:], -1.0, None, Al.is_le)
            s1bt = s1[:].rearrange("p (t hb) -> p hb t", hb=32)

            # ---- conv2 (2-chunk contraction over layer-1 halves) ----
            for bi in range(NBLK):
                ps = pp.tile([128, BLK], f32, name="cps")
                for c in range(2):
                    nc.tensor.matmul(
                        ps[:],
                        wsb[:, M2OFF + c * 128: M2OFF + (c + 1) * 128],
                        s1bt[:, c * 16 + 4 * bi: c * 16 + 4 * bi + 4, :],
                        start=(c == 0), stop=(c == 1))
                vt = big.tile([128, BLK], f32, name="vtmp", bufs=4)
                nc.vector.tensor_tensor_scan(
                    vt[:], al[:], ps[:], 0.0, Al.mult, Al.add)
                vdst = v2bt[:, 4 * bi: 4 * bi + 4, :]
                nc.scalar.copy(
                    vdst, vt[:].rearrange("p (b t) -> p b t", t=T))

            # ---- LIF layer 2: one fused custom-DVE op per step ----
            for t in range(T):
                nc.vector._custom_dve(
                    lif_op,
                    out=wp2[:, t * BL:(t + 1) * BL],
                    in0=(z0[:, 0:BL] if t == 0
                         else wp2[:, (t - 1) * BL:t * BL]),
                    in1=v2[:, t * BL:(t + 1) * BL],
                    s0=ALPHA, s1=-1.0)
            nc.vector.tensor_scalar(s2[:], wp2[:], -1.0, None, Al.is_le)
            s2bt = s2[:].rearrange("p (t b) -> p b t", b=BL)

            # ---- linear head ----
            for bi in range(NBLK):
                ps = pl.tile([2, BLK], f32, name="lps")
                nc.tensor.matmul(ps[:],
                                 wsb[:, LINOFF:LINOFF + 2],
                                 s2bt[:, 4 * bi:4 * bi + 4, :],
                                 start=True, stop=True)
                nc.scalar.copy(outsb[:, bi * BLK:(bi + 1) * BLK], ps[:])
                nc.sync.dma_start(out[:, bi * BLK:(bi + 1) * BLK],
                                  outsb[:, bi * BLK:(bi + 1) * BLK])

    return nc


_last_results = None


def _bass_forward(x, conv1_w, conv2_w, lin_w):
    global _last_results
    from concourse import bass_utils

    M1, M2 = _fused_mats(conv1_w, conv2_w)
    perm = _band_perm()
    M1p = M1[perm]                           # band-ordered output rows
    wband = np.empty((128, 20 * 128), np.float32)
    for idx, (j2, c) in enumerate(BANDS):
        wband[:, idx * 128:(idx + 1) * 128] = \
            M1p[j2 * 128:(j2 + 1) * 128, c * 128:(c + 1) * 128].T
    m2t = M2.T[perm].reshape(2, 128, 128).transpose(1, 0, 2).reshape(128, 256)
    lint = lin_w.T.astype(np.float32)        # [128, 2]
    wtsn = np.ascontiguousarray(np.concatenate(
        [wband, m2t.astype(np.float32), lint], axis=1))  # [128, 2818]

    nc = _build_bass()
    nc.finalize()  # runs Bacc.compile: matmul-wait moves + event-sem split
    in_maps = []
    for cid in range(NCORES):
        xs = x[cid * BL:(cid + 1) * BL].reshape(BT, 2048)
        # [quarter, img(400), c, p] -> [quarter, p, c(reordered), img]
        xb = np.ascontiguousarray(
            xs.reshape(NBLK, BLK, 16, 128)[:, :, CH_ORDER, :]
            .transpose(0, 3, 2, 1)).astype(np.float32)
        in_maps.append({"xt": xb, "wts": wtsn})
    res = bass_utils.run_bass_kernel_spmd(
        nc, in_maps, core_ids=list(range(NCORES)), trace=True)
    _last_results = res
    outp = np.empty((B, T, 2), np.float32)
    for cid in range(NCORES):
        o = res.results[cid]["out"]  # [2, 1600]
        outp[cid * BL:(cid + 1) * BL] = np.asarray(o, np.float32).reshape(
            2, BL, T).transpose(1, 2, 0)
    return outp


def kernel(x, conv1_w, conv2_w, lin_w):
    x = np.asarray(x, np.float32)
    conv1_w = np.asarray(conv1_w, np.float32)
    conv2_w = np.asarray(conv2_w, np.float32)
    lin_w = np.asarray(lin_w, np.float32)
    try:
        return _bass_forward(x, conv1_w, conv2_w, lin_w)
    except Exception as e:  # fall back to exact host computation
        import traceback
        traceback.print_exc()
        print(f"[kernel] bass path failed ({e!r}); using host fallback")
        return _numpy_forward(x, conv1_w, conv2_w, lin_w)


# revision 26
# speedup vs baseline: 1.1253x; 1.1253x over previous
import numpy as np

TAU = 10.0
THR = 1.0
ALPHA = float(np.exp(-1.0 / TAU))
B, T = 128, 100
NCORES = 8
BL = B // NCORES  # 16 samples per core
BT = BL * T       # 1600 images per core
NBLK = 4
BLK = BT // NBLK  # 400 images (4 samples) per block

# conv matmul mode: "f32r" (full speed, reduced precision) or "f32" (4x
# slower, exact). The net is chaotic (spike-time cascades amplify noise
# ~noise^0.46), so conv precision must be >=17 mantissa bits.
CONV_MODE = "f32"


def _fused_mats(conv1_w, conv2_w):
    """Fold avgpool into conv -> dense per-image matrices.
    M1: [256=(o,H,W), 2048=(i,h,w)]  (conv1 5x5 pad2 + pool4 -> 8x8 stride4 kernel, pad 2)
    M2: [128=(o2,H2,W2), 256=(i2,h,w)] (conv2 3x3 pad1 + pool2 -> 4x4 stride2 kernel, pad 1)
    """
    w1 = conv1_w.astype(np.float64)
    W1f = np.zeros((4, 2, 8, 8))
    for dh in range(4):
        for dw in range(4):
            W1f[:, :, dh:dh + 5, dw:dw + 5] += w1
    W1f /= 16.0
    M1 = np.zeros((4, 8, 8, 2, 32, 32))
    for Ho in range(8):
        for Wo in range(8):
            for a in range(8):
                h = 4 * Ho + a - 2
                if not (0 <= h < 32):
                    continue
                for b in range(8):
                    w = 4 * Wo + b - 2
                    if 0 <= w < 32:
                        M1[:, Ho, Wo, :, h, w] = W1f[:, :, a, b]
    M1 = M1.reshape(256, 2048).astype(np.float32)

    w2 = conv2_w.astype(np.float64)
    W2f = np.zeros((8, 4, 4, 4))
    for dh in range(2):
        for dw in range(2):
            W2f[:, :, dh:dh + 3, dw:dw + 3] += w2
    W2f /= 4.0
    M2 = np.zeros((8, 4, 4, 4, 8, 8))
    for Ho in range(4):
        for Wo in range(4):
            for a in range(4):
                h = 2 * Ho + a - 1
                if not (0 <= h < 8):
                    continue
                for b in range(4):
                    w = 2 * Wo + b - 1
                    if 0 <= w < 8:
                        M2[:, Ho, Wo, :, h, w] = W2f[:, :, a, b]
    M2 = M2.reshape(128, 256).astype(np.float32)
    return M1, M2


def _numpy_forward(x, conv1_w, conv2_w, lin_w):
    M1, M2 = _fused_mats(conv1_w, conv2_w)
    X = x.reshape(B * T, 2048).astype(np.float32)
    U1 = (X @ M1.T).reshape(B, T, 256)

    def leak_lif(U):  # U: [B,T,F] -> spikes [B,T,F]
        Bb, Tt, F = U.shape
        u = np.zeros((Bb, F), np.float32)
        v = np.zeros((Bb, F), np.float32)
        S = np.zeros_like(U)
        for t in range(Tt):
            u = ALPHA * u + U[:, t]
            v = ALPHA * v + u
            s = (v >= THR).astype(np.float32)
            v = v - s * THR
            S[:, t] = s
        return S

    S1 = leak_lif(U1)
    U2 = (S1.reshape(B * T, 256) @ M2.T).reshape(B, T, 128)
    S2 = leak_lif(U2)
    return (S2.reshape(B * T, 128) @ lin_w.T.astype(np.float32)).reshape(B, T, 2)


# conv1 band decomposition: output features are stored band-permuted
# (fnew = (Ho//4)*128 + (Ho%4)*32 + o*8 + Wo); each x k-chunk then feeds
# only one/two 128-row output halves, so conv1 is 20 matmuls per image
# block instead of 32. The permutation is absorbed into M2's column
# order on the host. BANDS is structural (conv geometry), not data.
BANDS = [(0, 0), (0, 1), (0, 2), (0, 3), (0, 4),
         (0, 8), (0, 9), (0, 10), (0, 11), (0, 12),
         (1, 3), (1, 4), (1, 5), (1, 6), (1, 7),
         (1, 11), (1, 12), (1, 13), (1, 14), (1, 15)]
WB = 20 * 128   # one band-weight set (bf16 hi or lo)


# x chunk order: the 10 chunks used by the j2=0 psum half come first, so
# conv1's first matmul group only waits for the first ~2MB of each quarter
CH_ORDER = [0, 1, 2, 3, 4, 8, 9, 10, 11, 12, 5, 6, 7, 13, 14, 15]
CH_POS = {c: i for i, c in enumerate(CH_ORDER)}


def _band_perm():
    perm = np.empty(256, np.int64)
    for Ho in range(8):
        for o in range(4):
            for Wo in range(8):
                fnew = (Ho // 4) * 128 + (Ho % 4) * 32 + o * 8 + Wo
                perm[fnew] = o * 64 + Ho * 8 + Wo
    return perm



_LIF_OP = None


def _get_lif_op():
    """Register a custom DVE op computing one LIF step in a single
    instruction:  out = (in0 + (in0 <= s1)) * s0 - in1
    (in0 = previous w_pre, in1 = leaked drive v[t], s0 = alpha, s1 = -1).
    4 ALU stages; replaces two chained scalar_tensor_tensor ops."""
    global _LIF_OP
    if _LIF_OP is not None:
        return _LIF_OP
    import numpy as np
    import concourse.dve_ops as dve_ops
    from concourse.dve_spec import Spec, Src0, Src1, C0, C1, lower, _has_src1
    from concourse.dve_uop import DveOpSpec

    name = "LIF_STEP_ANT"
    spec = Spec(
        body=(Src0 + (Src0 <= C1)) * C0 - Src1,
        reference=lambda in0, in1, s0, s1, imm2:
            (in0 + (in0 <= s1)) * s0 - in1,
    )
    row = dve_ops._CUSTOM_DVE_ROW_BASE + len(dve_ops.OPS)
    shas = {}
    for ver in ("v3", "v4"):
        try:
            dspec = DveOpSpec(name=name, opcode=row,
                              uops=lower(spec, ver=ver),
                              rd1_en=_has_src1(spec))
            shas[ver] = dspec.sha(ver)
        except Exception:
            pass
    op = dve_ops.DveOp(name, spec, subdim=False, uops_sha=shas)
    dve_ops.OPS.append(op)
    dve_ops._SUB_OPCODE_FOR_NAME[name] = row
    dve_ops.CUSTOM_DVE_SPECS[name] = spec
    _LIF_OP = op
    return op


def _build_bass():
    import concourse.mybir as mybir
    from concourse.bacc import Bacc
    from concourse.tile import TileContext

    f32 = mybir.dt.float32
    cdt = mybir.dt.float32r if CONV_MODE == "f32r" else mybir.dt.float32
    Al = mybir.AluOpType
    # Bacc (not bass.Bass): its finalize() runs move_matmul_waits_to_ldweights
    # + generate_event_semaphores, which legalize multi-sem waits down to the
    # 1-wait-per-instruction limit walrus enforces.
    lif_op = _get_lif_op()
    nc = Bacc()
    # x quarters: [4][p(k%128), c(k//128), img(400)] f32 — small enough that
    # conv1 on block 0 starts ~10us in
    bf16 = mybir.dt.bfloat16
    # x quarters, bf16 hi/lo pairs: [4][p, c(16 reordered), hl(2), img(400)]
    xt = nc.dram_tensor("xt", [NBLK, 128, 16, 2, BLK], bf16,
                        kind="ExternalInput")
    # conv1 band weights (bf16 hi then lo), conv2/lin weights (f32)
    wtsb = nc.dram_tensor("wtsb", [128, 2 * WB], bf16, kind="ExternalInput")
    wtsf = nc.dram_tensor("wtsf", [128, 258], cdt, kind="ExternalInput")
    out = nc.dram_tensor("out", [2, BT], f32, kind="ExternalOutput")

    with TileContext(nc) as tc:
        with (
            tc.tile_pool(name="big", bufs=1) as big,
            tc.tile_pool(name="xp", bufs=2) as xp,
            tc.tile_pool(name="pp", bufs=4, space="PSUM") as pp,
            tc.tile_pool(name="pl", bufs=2, space="PSUM") as pl,
        ):
            wsb = big.tile([128, 2 * WB], bf16, name="wsb")
            nc.sync.dma_start(wsb[:], wtsb[:, :])
            wsbf = big.tile([128, 258], cdt, name="wsbf")
            nc.sync.dma_start(wsbf[:], wtsf[:, :])

            xq = []
            for j in range(NBLK):
                tl = xp.tile([128, 16 * 2 * BLK], bf16, name="xblk")
                t3 = tl[:].rearrange("p (c h n) -> p c h n", h=2, n=BLK)
                nc.sync.dma_start(t3[:, 0:10], xt[j][:, 0:10])
                nc.sync.dma_start(t3[:, 10:16], xt[j][:, 10:16])
                xq.append(tl)

            # alpha pattern for leak scans: ALPHA everywhere, 0 at t%T==0
            # (DVE-built; the scans run on DVE so this is a same-engine dep)
            al = big.tile([128, BLK], f32, name="alpha")
            nc.vector.memset(al[:], ALPHA)
            al3 = al[:].rearrange("p (b t) -> p b t", t=T)
            nc.vector.memset(al3[:, :, 0], 0.0)

            # time-major state tiles: column index = t*LANES + lane, so the
            # per-step LIF slices are contiguous [128, LANES] (fast DVE path)
            v1 = big.tile([128, 2 * BT], f32, name="v1")    # [p, (t hb)]
            wp1 = big.tile([128, 2 * BT], f32, name="wp1")  # [p, (t hb)]
            s1 = big.tile([128, 2 * BT], cdt, name="s1")    # [p, (h b t)] b-major
            v2 = big.tile([128, BT], f32, name="v2")        # [p, (t b)]
            wp2 = big.tile([128, BT], f32, name="wp2")      # [p, (t b)]
            s2 = big.tile([128, BT], cdt, name="s2")        # [p, (b t)] b-major
            outsb = big.tile([2, BT], f32, name="outsb")

            # dummy PSUM tile: a tiny throwaway matmul absorbs each x-DMA
            # wait so real matmuls carry at most one sync wait each.
            dps = pl.tile([1, 8], f32, name="dps", bufs=1)

            # b-major scan landing pads (scan needs 2D APs); ACT then
            # relayouts them into the t-major state tiles
            vtmp_pool = xp  # reuse pool object only for namespacing

            # [p, hb(32) stride 1, t(100) stride 32] view of t-major layer-1
            v1bt = v1[:].rearrange("p (t hb) -> p hb t", hb=32)
            v2bt = v2[:].rearrange("p (t b) -> p b t", b=BL)

            # ---- conv1: dense fused conv+pool, 16-chunk contraction ----
            for bi in range(NBLK):
                xj = xq[bi][:, 0:1]
                nc.tensor.matmul(dps[:1, 0:1], xj, xj, start=True, stop=True)
                for h in range(2):
                    sub = [(i, c) for i, (jj, c) in enumerate(BANDS)
                           if jj == h]
                    ps = pp.tile([128, BLK], f32, name="cps")
                    nmm = 3 * len(sub)
                    k = 0
                    for idx, c in sub:
                        xh = xq[bi][:, (2 * CH_POS[c]) * BLK:
                                    (2 * CH_POS[c] + 1) * BLK]
                        xl = xq[bi][:, (2 * CH_POS[c] + 1) * BLK:
                                    (2 * CH_POS[c] + 2) * BLK]
                        wh = wsb[:, idx * 128:(idx + 1) * 128]
                        wl = wsb[:, WB + idx * 128: WB + (idx + 1) * 128]
                        for lhsT, rhs in ((wh, xh), (wh, xl), (wl, xh)):
                            nc.tensor.matmul(ps[:], lhsT, rhs,
                                             start=(k == 0),
                                             stop=(k == nmm - 1))
                            k += 1
                    # leak scan straight out of PSUM (per-sample reset via
                    # the alpha=0 columns) into a b-major temp, then ACT
                    # relayouts into t-major v1
                    vt = big.tile([128, BLK], f32, name="vtmp", bufs=4)
                    nc.vector.tensor_tensor_scan(
                        vt[:], al[:], ps[:], 0.0, Al.mult, Al.add)
                    vdst = v1bt[:, h * 16 + 4 * bi: h * 16 + 4 * bi + 4, :]
                    nc.scalar.copy(
                        vdst, vt[:].rearrange("p (b t) -> p b t", t=T))

            # ---- LIF layer 1: one fused custom-DVE op per step ----
            z0 = big.tile([128, 32], f32, name="z0")
            nc.vector.memset(z0[:], 0.0)
            for t in range(T):
                nc.vector._custom_dve(
                    lif_op,
                    out=wp1[:, t * 32:(t + 1) * 32],
                    in0=(z0[:] if t == 0 else wp1[:, (t - 1) * 32:t * 32]),
                    in1=v1[:, t * 32:(t + 1) * 32],
                    s0=ALPHA, s1=-1.0)
            # spikes: s = (w_pre <= -1), written per-half into b-major s1
            for h in range(2):
                src = wp1[:].rearrange("p (t hb) -> p t hb", hb=32)[
                    :, :, h * 16:(h + 1) * 16]              # (t, b) str (32,1)
                dst = s1[:, h * BT:(h + 1) * BT].rearrange(
                    "p (b t) -> p t b", t=T)                # (t, b) str (1,100)
                nc.vector.tensor_scalar(dst, src, -1.0, None, Al.is_le)

            # ---- conv2 (2-chunk contraction over layer-1 halves) ----
            for bi in range(NBLK):
                ps = pp.tile([128, BLK], f32, name="cps")
                for c in range(2):
                    nc.tensor.matmul(
                        ps[:],
                        wsbf[:, c * 128:(c + 1) * 128],
                        s1[:, c * BT + bi * BLK: c * BT + (bi + 1) * BLK],
                        start=(c == 0), stop=(c == 1))
                vt = big.tile([128, BLK], f32, name="vtmp", bufs=4)
                nc.vector.tensor_tensor_scan(
                    vt[:], al[:], ps[:], 0.0, Al.mult, Al.add)
                vdst = v2bt[:, 4 * bi: 4 * bi + 4, :]
                nc.scalar.copy(
                    vdst, vt[:].rearrange("p (b t) -> p b t", t=T))

            # ---- LIF layer 2: one fused custom-DVE op per step ----
            for t in range(T):
                nc.vector._custom_dve(
                    lif_op,
                    out=wp2[:, t * BL:(t + 1) * BL],
                    in0=(z0[:, 0:BL] if t == 0
                         else wp2[:, (t - 1) * BL:t * BL]),
                    in1=v2[:, t * BL:(t + 1) * BL],
                    s0=ALPHA, s1=-1.0)
            src2 = wp2[:].rearrange("p (t b) -> p t b", b=BL)
            dst2 = s2[:].rearrange("p (b t) -> p t b", t=T)
            nc.vector.tensor_scalar(dst2, src2, -1.0, None, Al.is_le)

            # ---- linear head ----
            for bi in range(NBLK):
                ps = pl.tile([2, BLK], f32, name="lps")
                nc.tensor.matmul(ps[:],
                                 wsbf[:, 256:258],
                                 s2[:, bi * BLK:(bi + 1) * BLK],
                                 start=True, stop=True)
                nc.scalar.copy(outsb[:, bi * BLK:(bi + 1) * BLK], ps[:])
                nc.sync.dma_start(out[:, bi * BLK:(bi + 1) * BLK],
                                  outsb[:, bi * BLK:(bi + 1) * BLK])

    return nc


_last_results = None


def _bass_forward(x, conv1_w, conv2_w, lin_w):
    global _last_results
    from concourse import bass_utils

    import ml_dtypes
    bf16 = ml_dtypes.bfloat16
    M1, M2 = _fused_mats(conv1_w, conv2_w)
    perm = _band_perm()
    M1p = M1[perm]                           # band-ordered output rows
    wband = np.empty((128, 20 * 128), np.float32)
    for idx, (j2, c) in enumerate(BANDS):
        wband[:, idx * 128:(idx + 1) * 128] = \
            M1p[j2 * 128:(j2 + 1) * 128, c * 128:(c + 1) * 128].T
    wh = wband.astype(bf16)
    wlo = (wband - wh.astype(np.float32)).astype(bf16)
    wtsb_n = np.ascontiguousarray(np.concatenate([wh, wlo], axis=1))
    m2t = M2.T[perm].reshape(2, 128, 128).transpose(1, 0, 2).reshape(128, 256)
    lint = lin_w.T.astype(np.float32)        # [128, 2]
    wtsf_n = np.ascontiguousarray(np.concatenate(
        [m2t.astype(np.float32), lint], axis=1))  # [128, 258]

    nc = _build_bass()
    nc.finalize()  # runs Bacc.compile: matmul-wait moves + event-sem split
    in_maps = []
    for cid in range(NCORES):
        xs = x[cid * BL:(cid + 1) * BL].reshape(BT, 2048)
        # [quarter, img, c, p] -> bf16 hi/lo -> [quarter, p, c(reord), hl, img]
        x4 = xs.reshape(NBLK, BLK, 16, 128)[:, :, CH_ORDER, :]
        xh4 = x4.astype(bf16)
        xl4 = (x4 - xh4.astype(np.float32)).astype(bf16)
        xb = np.ascontiguousarray(
            np.stack([xh4, xl4], axis=3).transpose(0, 4, 2, 3, 1))
        in_maps.append({"xt": xb, "wtsb": wtsb_n, "wtsf": wtsf_n})
    res = bass_utils.run_bass_kernel_spmd(
        nc, in_maps, core_ids=list(range(NCORES)), trace=True)
    _last_results = res
    outp = np.empty((B, T, 2), np.float32)
    for cid in range(NCORES):
        o = res.results[cid]["out"]  # [2, 1600]
        outp[cid * BL:(cid + 1) * BL] = np.asarray(o, np.float32).reshape(
            2, BL, T).transpose(1, 2, 0)
    return outp


def kernel(x, conv1_w, conv2_w, lin_w):
    x = np.asarray(x, np.float32)
    conv1_w = np.asarray(conv1_w, np.float32)
    conv2_w = np.asarray(conv2_w, np.float32)
    lin_w = np.asarray(lin_w, np.float32)
    try:
        return _bass_forward(x, conv1_w, conv2_w, lin_w)
    except Exception as e:  # fall back to exact host computation
        import traceback
        traceback.print_exc()
        print(f"[kernel] bass path failed ({e!r}); using host fallback")
        return _numpy_forward(x, conv1_w, conv2_w, lin_w)


# revision 27
# speedup vs baseline: 1.1331x; 1.0070x over previous
import numpy as np

TAU = 10.0
THR = 1.0
ALPHA = float(np.exp(-1.0 / TAU))
B, T = 128, 100
NCORES = 8
BL = B // NCORES  # 16 samples per core
BT = BL * T       # 1600 images per core
NBLK = 4
BLK = BT // NBLK  # 400 images (4 samples) per block

# conv matmul mode: "f32r" (full speed, reduced precision) or "f32" (4x
# slower, exact). The net is chaotic (spike-time cascades amplify noise
# ~noise^0.46), so conv precision must be >=17 mantissa bits.
CONV_MODE = "f32"


def _fused_mats(conv1_w, conv2_w):
    """Fold avgpool into conv -> dense per-image matrices.
    M1: [256=(o,H,W), 2048=(i,h,w)]  (conv1 5x5 pad2 + pool4 -> 8x8 stride4 kernel, pad 2)
    M2: [128=(o2,H2,W2), 256=(i2,h,w)] (conv2 3x3 pad1 + pool2 -> 4x4 stride2 kernel, pad 1)
    """
    w1 = conv1_w.astype(np.float64)
    W1f = np.zeros((4, 2, 8, 8))
    for dh in range(4):
        for dw in range(4):
            W1f[:, :, dh:dh + 5, dw:dw + 5] += w1
    W1f /= 16.0
    M1 = np.zeros((4, 8, 8, 2, 32, 32))
    for Ho in range(8):
        for Wo in range(8):
            for a in range(8):
                h = 4 * Ho + a - 2
                if not (0 <= h < 32):
                    continue
                for b in range(8):
                    w = 4 * Wo + b - 2
                    if 0 <= w < 32:
                        M1[:, Ho, Wo, :, h, w] = W1f[:, :, a, b]
    M1 = M1.reshape(256, 2048).astype(np.float32)

    w2 = conv2_w.astype(np.float64)
    W2f = np.zeros((8, 4, 4, 4))
    for dh in range(2):
        for dw in range(2):
            W2f[:, :, dh:dh + 3, dw:dw + 3] += w2
    W2f /= 4.0
    M2 = np.zeros((8, 4, 4, 4, 8, 8))
    for Ho in range(4):
        for Wo in range(4):
            for a in range(4):
                h = 2 * Ho + a - 1
                if not (0 <= h < 8):
                    continue
                for b in range(4):
                    w = 2 * Wo + b - 1
                    if 0 <= w < 8:
                        M2[:, Ho, Wo, :, h, w] = W2f[:, :, a, b]
    M2 = M2.reshape(128, 256).astype(np.float32)
    return M1, M2


def _numpy_forward(x, conv1_w, conv2_w, lin_w):
    M1, M2 = _fused_mats(conv1_w, conv2_w)
    X = x.reshape(B * T, 2048).astype(np.float32)
    U1 = (X @ M1.T).reshape(B, T, 256)

    def leak_lif(U):  # U: [B,T,F] -> spikes [B,T,F]
        Bb, Tt, F = U.shape
        u = np.zeros((Bb, F), np.float32)
        v = np.zeros((Bb, F), np.float32)
        S = np.zeros_like(U)
        for t in range(Tt):
            u = ALPHA * u + U[:, t]
            v = ALPHA * v + u
            s = (v >= THR).astype(np.float32)
            v = v - s * THR
            S[:, t] = s
        return S

    S1 = leak_lif(U1)
    U2 = (S1.reshape(B * T, 256) @ M2.T).reshape(B, T, 128)
    S2 = leak_lif(U2)
    return (S2.reshape(B * T, 128) @ lin_w.T.astype(np.float32)).reshape(B, T, 2)


# conv1 band decomposition: output features are stored band-permuted
# (fnew = (Ho//4)*128 + (Ho%4)*32 + o*8 + Wo); each x k-chunk then feeds
# only one/two 128-row output halves, so conv1 is 20 matmuls per image
# block instead of 32. The permutation is absorbed into M2's column
# order on the host. BANDS is structural (conv geometry), not data.
BANDS = [(0, 0), (0, 1), (0, 2), (0, 3), (0, 4),
         (0, 8), (0, 9), (0, 10), (0, 11), (0, 12),
         (1, 3), (1, 4), (1, 5), (1, 6), (1, 7),
         (1, 11), (1, 12), (1, 13), (1, 14), (1, 15)]
WB = 20 * 128   # one band-weight set (bf16 hi or lo)


# x chunk order: the 10 chunks used by the j2=0 psum half come first, so
# conv1's first matmul group only waits for the first ~2MB of each quarter
CH_ORDER = [0, 1, 2, 3, 4, 8, 9, 10, 11, 12, 5, 6, 7, 13, 14, 15]
CH_POS = {c: i for i, c in enumerate(CH_ORDER)}


def _band_perm():
    perm = np.empty(256, np.int64)
    for Ho in range(8):
        for o in range(4):
            for Wo in range(8):
                fnew = (Ho // 4) * 128 + (Ho % 4) * 32 + o * 8 + Wo
                perm[fnew] = o * 64 + Ho * 8 + Wo
    return perm



_LIF_OP = None


def _get_lif_op():
    """Register a custom DVE op computing one LIF step in a single
    instruction:  out = (in0 + (in0 <= s1)) * s0 - in1
    (in0 = previous w_pre, in1 = leaked drive v[t], s0 = alpha, s1 = -1).
    4 ALU stages; replaces two chained scalar_tensor_tensor ops."""
    global _LIF_OP
    if _LIF_OP is not None:
        return _LIF_OP
    import numpy as np
    import concourse.dve_ops as dve_ops
    from concourse.dve_spec import Spec, Src0, Src1, C0, C1, lower, _has_src1
    from concourse.dve_uop import DveOpSpec

    name = "LIF_STEP_ANT"
    spec = Spec(
        body=(Src0 + (Src0 <= C1)) * C0 - Src1,
        reference=lambda in0, in1, s0, s1, imm2:
            (in0 + (in0 <= s1)) * s0 - in1,
    )
    row = dve_ops._CUSTOM_DVE_ROW_BASE + len(dve_ops.OPS)
    shas = {}
    for ver in ("v3", "v4"):
        try:
            dspec = DveOpSpec(name=name, opcode=row,
                              uops=lower(spec, ver=ver),
                              rd1_en=_has_src1(spec))
            shas[ver] = dspec.sha(ver)
        except Exception:
            pass
    op = dve_ops.DveOp(name, spec, subdim=False, uops_sha=shas)
    dve_ops.OPS.append(op)
    dve_ops._SUB_OPCODE_FOR_NAME[name] = row
    dve_ops.CUSTOM_DVE_SPECS[name] = spec
    _LIF_OP = op
    return op


def _build_bass():
    import concourse.mybir as mybir
    from concourse.bacc import Bacc
    from concourse.tile import TileContext

    f32 = mybir.dt.float32
    cdt = mybir.dt.float32r if CONV_MODE == "f32r" else mybir.dt.float32
    Al = mybir.AluOpType
    # Bacc (not bass.Bass): its finalize() runs move_matmul_waits_to_ldweights
    # + generate_event_semaphores, which legalize multi-sem waits down to the
    # 1-wait-per-instruction limit walrus enforces.
    lif_op = _get_lif_op()
    nc = Bacc()
    # x quarters: [4][p(k%128), c(k//128), img(400)] f32 — small enough that
    # conv1 on block 0 starts ~10us in
    bf16 = mybir.dt.bfloat16
    # x quarters, bf16 hi/lo pairs: [4][p, c(16 reordered), hl(2), img(400)]
    xt = nc.dram_tensor("xt", [NBLK, 128, 16, 2, BLK], bf16,
                        kind="ExternalInput")
    # conv1 band weights (bf16 hi then lo), conv2/lin weights (f32)
    # wtsb: conv1 band weights (hi, lo) then lin hi/lo [128, 4]
    wtsb = nc.dram_tensor("wtsb", [128, 2 * WB + 4], bf16,
                          kind="ExternalInput")
    wtsf = nc.dram_tensor("wtsf", [128, 256], cdt, kind="ExternalInput")
    out = nc.dram_tensor("out", [2, BT], f32, kind="ExternalOutput")

    with TileContext(nc) as tc:
        with (
            tc.tile_pool(name="big", bufs=1) as big,
            tc.tile_pool(name="xp", bufs=2) as xp,
            tc.tile_pool(name="pp", bufs=4, space="PSUM") as pp,
            tc.tile_pool(name="pl", bufs=2, space="PSUM") as pl,
        ):
            xq = []
            xtiles = []
            for j in range(NBLK):
                tl = xp.tile([128, 16 * 2 * BLK], bf16, name="xblk")
                xq.append(tl)
            # q0's first chunk-group goes out first so conv1 block 0 can
            # start ASAP; weights (small) next; the rest follow
            t3 = xq[0][:].rearrange("p (c h n) -> p c h n", h=2, n=BLK)
            nc.sync.dma_start(t3[:, 0:10], xt[0][:, 0:10])
            wsb = big.tile([128, 2 * WB + 4], bf16, name="wsb")
            nc.sync.dma_start(wsb[:], wtsb[:, :])
            wsbf = big.tile([128, 256], cdt, name="wsbf")
            nc.sync.dma_start(wsbf[:], wtsf[:, :])
            nc.sync.dma_start(t3[:, 10:16], xt[0][:, 10:16])
            for j in range(1, NBLK):
                t3 = xq[j][:].rearrange("p (c h n) -> p c h n", h=2, n=BLK)
                nc.sync.dma_start(t3[:, 0:10], xt[j][:, 0:10])
                nc.sync.dma_start(t3[:, 10:16], xt[j][:, 10:16])

            # alpha pattern for leak scans: ALPHA everywhere, 0 at t%T==0
            # (DVE-built; the scans run on DVE so this is a same-engine dep)
            al = big.tile([128, BLK], f32, name="alpha")
            nc.vector.memset(al[:], ALPHA)
            al3 = al[:].rearrange("p (b t) -> p b t", t=T)
            nc.vector.memset(al3[:, :, 0], 0.0)

            # time-major state tiles: column index = t*LANES + lane, so the
            # per-step LIF slices are contiguous [128, LANES] (fast DVE path)
            v1 = big.tile([128, 2 * BT], f32, name="v1")    # [p, (t hb)]
            wp1 = big.tile([128, 2 * BT], f32, name="wp1")  # [p, (t hb)]
            s1 = big.tile([128, 2 * BT], cdt, name="s1")    # [p, (h b t)] b-major
            v2 = big.tile([128, BT], f32, name="v2")        # [p, (t b)]
            wp2 = big.tile([128, BT], f32, name="wp2")      # [p, (t b)]
            s2 = big.tile([128, BT], bf16, name="s2")       # [p, (b t)] b-major
            outsb = big.tile([2, BT], f32, name="outsb")

            # dummy PSUM tile: a tiny throwaway matmul absorbs each x-DMA
            # wait so real matmuls carry at most one sync wait each.
            dps = pl.tile([1, 8], f32, name="dps", bufs=1)

            # b-major scan landing pads (scan needs 2D APs); ACT then
            # relayouts them into the t-major state tiles
            vtmp_pool = xp  # reuse pool object only for namespacing

            # [p, hb(32) stride 1, t(100) stride 32] view of t-major layer-1
            v1bt = v1[:].rearrange("p (t hb) -> p hb t", hb=32)
            v2bt = v2[:].rearrange("p (t b) -> p b t", b=BL)

            # ---- conv1: dense fused conv+pool, 16-chunk contraction ----
            for bi in range(NBLK):
                xj = xq[bi][:, 0:1]
                nc.tensor.matmul(dps[:1, 0:1], xj, xj, start=True, stop=True)
                for h in range(2):
                    sub = [(i, c) for i, (jj, c) in enumerate(BANDS)
                           if jj == h]
                    ps = pp.tile([128, BLK], f32, name="cps")
                    nmm = 3 * len(sub)
                    k = 0
                    for idx, c in sub:
                        xh = xq[bi][:, (2 * CH_POS[c]) * BLK:
                                    (2 * CH_POS[c] + 1) * BLK]
                        xl = xq[bi][:, (2 * CH_POS[c] + 1) * BLK:
                                    (2 * CH_POS[c] + 2) * BLK]
                        wh = wsb[:, idx * 128:(idx + 1) * 128]
                        wl = wsb[:, WB + idx * 128: WB + (idx + 1) * 128]
                        for lhsT, rhs in ((wh, xh), (wh, xl), (wl, xh)):
                            nc.tensor.matmul(ps[:], lhsT, rhs,
                                             start=(k == 0),
                                             stop=(k == nmm - 1))
                            k += 1
                    # leak scan straight out of PSUM (per-sample reset via
                    # the alpha=0 columns) into a b-major temp, then ACT
                    # relayouts into t-major v1
                    vt = big.tile([128, BLK], f32, name="vtmp", bufs=4)
                    nc.vector.tensor_tensor_scan(
                        vt[:], al[:], ps[:], 0.0, Al.mult, Al.add)
                    vdst = v1bt[:, h * 16 + 4 * bi: h * 16 + 4 * bi + 4, :]
                    nc.scalar.copy(
                        vdst, vt[:].rearrange("p (b t) -> p b t", t=T))

            # ---- LIF layer 1: one fused custom-DVE op per step ----
            z0 = big.tile([128, 32], f32, name="z0")
            nc.vector.memset(z0[:], 0.0)
            for t in range(T):
                nc.vector._custom_dve(
                    lif_op,
                    out=wp1[:, t * 32:(t + 1) * 32],
                    in0=(z0[:] if t == 0 else wp1[:, (t - 1) * 32:t * 32]),
                    in1=v1[:, t * 32:(t + 1) * 32],
                    s0=ALPHA, s1=-1.0)
            # spikes: s = (w_pre <= -1), written per-half into b-major s1
            for h in range(2):
                src = wp1[:].rearrange("p (t hb) -> p t hb", hb=32)[
                    :, :, h * 16:(h + 1) * 16]              # (t, b) str (32,1)
                dst = s1[:, h * BT:(h + 1) * BT].rearrange(
                    "p (b t) -> p t b", t=T)                # (t, b) str (1,100)
                nc.vector.tensor_scalar(dst, src, -1.0, None, Al.is_le)

            # ---- conv2 (2-chunk contraction over layer-1 halves) ----
            for bi in range(NBLK):
                ps = pp.tile([128, BLK], f32, name="cps")
                for c in range(2):
                    nc.tensor.matmul(
                        ps[:],
                        wsbf[:, c * 128:(c + 1) * 128],
                        s1[:, c * BT + bi * BLK: c * BT + (bi + 1) * BLK],
                        start=(c == 0), stop=(c == 1))
                vt = big.tile([128, BLK], f32, name="vtmp", bufs=4)
                nc.vector.tensor_tensor_scan(
                    vt[:], al[:], ps[:], 0.0, Al.mult, Al.add)
                vdst = v2bt[:, 4 * bi: 4 * bi + 4, :]
                nc.scalar.copy(
                    vdst, vt[:].rearrange("p (b t) -> p b t", t=T))

            # ---- LIF layer 2: one fused custom-DVE op per step ----
            for t in range(T):
                nc.vector._custom_dve(
                    lif_op,
                    out=wp2[:, t * BL:(t + 1) * BL],
                    in0=(z0[:, 0:BL] if t == 0
                         else wp2[:, (t - 1) * BL:t * BL]),
                    in1=v2[:, t * BL:(t + 1) * BL],
                    s0=ALPHA, s1=-1.0)
            src2 = wp2[:].rearrange("p (t b) -> p t b", b=BL)
            dst2 = s2[:].rearrange("p (b t) -> p t b", t=T)
            nc.vector.tensor_scalar(dst2, src2, -1.0, None, Al.is_le)

            # ---- linear head ----
            for bi in range(NBLK):
                ps = pl.tile([2, BLK], f32, name="lps")
                nc.tensor.matmul(ps[:],
                                 wsb[:, 2 * WB:2 * WB + 2],
                                 s2[:, bi * BLK:(bi + 1) * BLK],
                                 start=True, stop=False)
                nc.tensor.matmul(ps[:],
                                 wsb[:, 2 * WB + 2:2 * WB + 4],
                                 s2[:, bi * BLK:(bi + 1) * BLK],
                                 start=False, stop=True)
                nc.scalar.copy(outsb[:, bi * BLK:(bi + 1) * BLK], ps[:])
                nc.sync.dma_start(out[:, bi * BLK:(bi + 1) * BLK],
                                  outsb[:, bi * BLK:(bi + 1) * BLK])

    return nc


_last_results = None


def _bass_forward(x, conv1_w, conv2_w, lin_w):
    global _last_results
    from concourse import bass_utils

    import ml_dtypes
    bf16 = ml_dtypes.bfloat16
    M1, M2 = _fused_mats(conv1_w, conv2_w)
    perm = _band_perm()
    M1p = M1[perm]                           # band-ordered output rows
    wband = np.empty((128, 20 * 128), np.float32)
    for idx, (j2, c) in enumerate(BANDS):
        wband[:, idx * 128:(idx + 1) * 128] = \
            M1p[j2 * 128:(j2 + 1) * 128, c * 128:(c + 1) * 128].T
    wh = wband.astype(bf16)
    wlo = (wband - wh.astype(np.float32)).astype(bf16)
    wtsb_n = np.ascontiguousarray(np.concatenate([wh, wlo], axis=1))
    m2t = M2.T[perm].reshape(2, 128, 128).transpose(1, 0, 2).reshape(128, 256)
    lint = lin_w.T.astype(np.float32)        # [128, 2]
    lh = lint.astype(bf16)
    ll = (lint - lh.astype(np.float32)).astype(bf16)
    wtsb_n = np.ascontiguousarray(np.concatenate([wh, wlo, lh, ll], axis=1))
    wtsf_n = np.ascontiguousarray(m2t.astype(np.float32))  # [128, 256]

    nc = _build_bass()
    nc.finalize()  # runs Bacc.compile: matmul-wait moves + event-sem split
    in_maps = []
    for cid in range(NCORES):
        xs = x[cid * BL:(cid + 1) * BL].reshape(BT, 2048)
        # [quarter, img, c, p] -> bf16 hi/lo -> [quarter, p, c(reord), hl, img]
        x4 = xs.reshape(NBLK, BLK, 16, 128)[:, :, CH_ORDER, :]
        xh4 = x4.astype(bf16)
        xl4 = (x4 - xh4.astype(np.float32)).astype(bf16)
        xb = np.ascontiguousarray(
            np.stack([xh4, xl4], axis=3).transpose(0, 4, 2, 3, 1))
        in_maps.append({"xt": xb, "wtsb": wtsb_n, "wtsf": wtsf_n})
    res = bass_utils.run_bass_kernel_spmd(
        nc, in_maps, core_ids=list(range(NCORES)), trace=True)
    _last_results = res
    outp = np.empty((B, T, 2), np.float32)
    for cid in range(NCORES):
        o = res.results[cid]["out"]  # [2, 1600]
        outp[cid * BL:(cid + 1) * BL] = np.asarray(o, np.float32).reshape(
            2, BL, T).transpose(1, 2, 0)
    return outp


def kernel(x, conv1_w, conv2_w, lin_w):
    x = np.asarray(x, np.float32)
    conv1_w = np.asarray(conv1_w, np.float32)
    conv2_w = np.asarray(conv2_w, np.float32)
    lin_w = np.asarray(lin_w, np.float32)
    try:
        return _bass_forward(x, conv1_w, conv2_w, lin_w)
    except Exception as e:  # fall back to exact host computation
        import traceback
        traceback.print_exc()
        print(f"[kernel] bass path failed ({e!r}); using host fallback")
        return _numpy_forward(x, conv1_w, conv2_w, lin_w)
